# revision 13
# baseline (speedup 1.0000x reference)
"""Trainium2 Bass kernel for an 8-expert top-2 MoE block (B=4, T=2048, C=1024, H=4C).

Strategy (hidden-dim tensor-parallel + mixed-precision routing):
  - Host computes the gate (logits -> top-2 -> softmax) and gathers each
    expert's routed tokens into padded token streams.
  - Each of the 8 NeuronCores holds a 512-wide slice of the hidden (H)
    dimension of ALL 8 experts' weights and processes EVERY routed token
    (perfectly load-balanced regardless of routing skew).
  - Mixed precision by gate weight: token-expert pairs whose top-2
    combine weight is below THETA run in fp8(e4m3) with
    perf_mode=DoubleRow (2x matmul throughput, measured 2.01x on HW);
    the rest run in bf16. The gate weight attenuates the fp8
    quantization error (~5.4% per expert output), so total rel err
    stays ~1.9e-2 < 2e-2, and each token's dominant expert (weight >=
    0.5) always stays bf16. 23% of pairs take the fp8 path.
  - Outputs are stored as bf16 (halves the store traffic; the 8
    H-slice partials are summed in fp32 on the host).
  - fp8 weights are pre-scaled by SW=1024 on the host (w ~ N(0,0.02)
    would be subnormal in e4m3); the descale folds into the gelu's
    input scale (layer 1) and the host combine (layer 2).
  - fp8 chunks' PSUM drains split across ScalarE+VectorE (the fp8
    phase-B matmuls are 2x faster, VectorE alone can't keep up).

All matmuls accumulate in fp32 PSUM; rel err vs the fp32 reference is
~1.91e-2 (dominated by the fp8 path, tunable via THETA). HW exec time
~417.5us vs the 469.5us all-bf16 baseline (the bf16 compute floor on
this problem is ~437us; fp8-DoubleRow on the low-weight pairs is what
goes below it).
"""

import sys

for _p in ("/opt/trn_rl_repo", "/root/.axon_site/_ro/trn_rl_repo"):
    if _p not in sys.path:
        sys.path.insert(0, _p)

from contextlib import ExitStack

import ml_dtypes
import numpy as np

import bass_rust
import concourse.bass as bass
import concourse.mybir as mybir
import concourse.tile as tile
from concourse.bass_utils import run_bass_kernel_spmd

B, T, C, E = 4, 2048, 1024, 8
H = 4 * C
N = B * T
TOP_K = 2
P = 128            # partitions
HS = H // 8        # per-core hidden slice (512)
CK = C // P        # 8 contraction chunks for x @ w1
LHK = HS // P      # 4 local h-groups per chunk
TOK = 512          # max token chunk (PSUM bank = 512 fp32)
MIN_TAIL = 192     # min bf16 tail chunk so LDWEIGHTS stays hidden
MIN_TAIL8 = 256    # min fp8 tail (DoubleRow LDWEIGHTS = 256 cols)
THETA = 0.43       # combine-weight threshold below which a pair runs fp8
SW = 1024.0        # fp8 weight pre-scale

BF16 = mybir.dt.bfloat16
F32 = mybir.dt.float32
F8 = mybir.dt.float8e4
E4NP = ml_dtypes.float8_e4m3fn


def _legalize_waits(nc: "bass.Bass") -> None:
    """Split multi-wait instructions into standalone EventSemaphore waits.

    The walrus build here accepts at most one sync-wait command per
    instruction (setupSyncWait "Too many sync wait commands"), but Tile
    attaches every outstanding dependency to the consuming instruction.
    Hoist all but the last wait onto same-engine EventSemaphore
    instructions placed immediately before the consumer: the engine's
    sequencer processes them in order, so the dependency still holds.
    """

    def fix_block(bb):
        out = []
        for inst in bb.instructions:
            si = inst.sync_info
            if si is not None and len(si.on_wait) > 1:
                waits = list(si.on_wait)
                for k, w in enumerate(waits[:-1]):
                    ev = bass_rust.InstEventSemaphore(
                        name=f"{inst.name}-lw{k}", ins=[], outs=[],
                        engine=inst.engine,
                    )
                    ev.sync_info = bass_rust.SyncInfo(on_wait=[w], on_update=[])
                    out.append(ev)
                inst.sync_info = bass_rust.SyncInfo(
                    on_wait=[waits[-1]], on_update=list(si.on_update)
                )
            out.append(inst)
        bb.instructions = out
        for sub in getattr(bb, "blocks", []) or []:
            fix_block(sub)

    for fn in nc.m.functions:
        for bb in fn.blocks:
            fix_block(bb)


def _ceil16(v: int) -> int:
    return -(-v // 16) * 16


def _ceil8(v: int) -> int:
    return -(-v // 8) * 8


def _expert_chunks(count: int) -> list[int]:
    """bf16 chunk sizes: full 512s plus one or two tails in
    [MIN_TAIL, 512], padded to a multiple of 8 (keeps every bf16 AP
    stride 16-byte aligned; the %16 element constraint only applies to
    the fp8 DoubleRow 3D APs)."""
    if count <= 0:
        return []
    k, r = divmod(count, TOK)
    if r == 0:
        return [TOK] * k
    if r >= MIN_TAIL or k == 0:
        return [TOK] * k + [max(_ceil8(r), MIN_TAIL if k else 16)]
    r += TOK
    t1 = max(_ceil8(r // 2), MIN_TAIL)
    t2 = max(_ceil8(r - t1), MIN_TAIL)
    return [TOK] * (k - 1) + [t1, t2]


def _expert_chunks8(count: int) -> list[int]:
    """fp8 chunk sizes: full 512s plus tails >= MIN_TAIL8 (DoubleRow
    LDWEIGHTS is 256 cols; smaller chunks go LDWEIGHTS-bound)."""
    if count <= 0:
        return []
    k, r = divmod(count, TOK)
    if r == 0:
        return [TOK] * k
    if r >= MIN_TAIL8:
        return [TOK] * k + [_ceil16(r)]
    if k == 0:
        return [_ceil16(max(r, MIN_TAIL8))]
    r += TOK
    t1 = _ceil16(r // 2)
    t2 = _ceil16(r - t1)
    return [TOK] * (k - 1) + [t1, t2]


def _build_nc(schedule, cap_bf: int, cap_8: int, cap_y: int) -> bass.Bass:
    """schedule: list of (expert, tok, x_off, y_off, kind) descriptors,
    kind 0 = bf16, 1 = fp8 DoubleRow.

    DRAM layouts (host pre-tiled, partition-major, cb-major per chunk):
      xT  [P, CK*cap_bf] bf16   bf16 chunk at cols CK*x_off
      x8T [P, CK*cap_8]  fp8    fp8 chunk at cols CK*x_off
      w1  [P, E*CK*HS]  bf16    expert e at cols e*CK*HS, cb-major
      w2  [P, E*LHK*C]  bf16    expert e at cols e*LHK*C, lh-major
      w1q [P, E*CK*HS]  fp8     same layout, values pre-scaled by SW
      w2q [P, E*LHK*C]  fp8     same layout, values pre-scaled by SW
      b1  [P, E*LHK]    f32     col g = b1 slice values for h-group g
      yT  [P, CK*cap_y] bf16    chunk at cols CK*y_off (co-major)
    """
    nc = bass.Bass()
    xT = nc.declare_dram_parameter("xT", [P, CK * cap_bf], BF16, isOutput=False)
    x8T = nc.declare_dram_parameter("x8T", [P, CK * cap_8], F8, isOutput=False)
    w1 = nc.declare_dram_parameter("w1", [P, E * CK * HS], BF16, isOutput=False)
    w2 = nc.declare_dram_parameter("w2", [P, E * LHK * C], BF16, isOutput=False)
    w1q = nc.declare_dram_parameter("w1q", [P, E * CK * HS], F8, isOutput=False)
    w2q = nc.declare_dram_parameter("w2q", [P, E * LHK * C], F8, isOutput=False)
    b1 = nc.declare_dram_parameter("b1", [P, E * LHK], F32, isOutput=False)
    yT = nc.declare_dram_parameter("yT", [P, CK * cap_y], BF16, isOutput=True)

    gelu = mybir.ActivationFunctionType.Gelu
    DR = mybir.MatmulPerfMode.DoubleRow

    has8 = [False] * E
    hasbf = [False] * E
    for e, tok, xo, yo, kind in schedule:
        if kind:
            has8[e] = True
        else:
            hasbf[e] = True

    with tile.TileContext(nc) as tc, ExitStack() as ctx:
        w1p = ctx.enter_context(tc.tile_pool(name="w1p", bufs=3))
        w2p = ctx.enter_context(tc.tile_pool(name="w2p", bufs=3))
        w1qp = ctx.enter_context(tc.tile_pool(name="w1qp", bufs=2))
        w2qp = ctx.enter_context(tc.tile_pool(name="w2qp", bufs=2))
        cst = ctx.enter_context(tc.tile_pool(name="cst", bufs=1))
        xp = ctx.enter_context(tc.tile_pool(name="xp", bufs=4))
        x8p = ctx.enter_context(tc.tile_pool(name="x8p", bufs=3))
        hp = ctx.enter_context(tc.tile_pool(name="hp", bufs=2 * LHK))
        h8p = ctx.enter_context(tc.tile_pool(name="h8p", bufs=2))
        op = ctx.enter_context(tc.tile_pool(name="op", bufs=4))
        psA = ctx.enter_context(tc.tile_pool(name="psA", bufs=4, space="PSUM"))
        psB = ctx.enter_context(tc.tile_pool(name="psB", bufs=4, space="PSUM"))

        # Warm the PE HAM clock gate while the first DMAs stream: small
        # (N=128) back-to-back matmuls from ~7.9us (right after the
        # GpSimd preamble frees the tiny memset) give ~4.5us of
        # continuous PE activity, so a fully-busy 4096-cycle HAM window
        # (the 2.4 GHz flip threshold) completes before the first real
        # chunk. Sized to end ~12.7us: chunk0's data lands ~12.6us
        # normally but the DGE queue start jitters several us late, and
        # a >3.4us PE-idle gap would re-throttle the clock -- ending a
        # bit later keeps the worst case warm at ~0.1us typical cost.
        NWARM = 44
        dummy = cst.tile([P, P], BF16, tag="dummy")
        nc.gpsimd.memset(dummy[:], 0.0)
        warm = psB.tile([P, P], F32, tag="psB", name="warm")
        for i in range(NWARM):
            nc.tensor.matmul(warm[:], dummy[:], dummy[:],
                             start=(i == 0), stop=(i == NWARM - 1))

        b1_sb = cst.tile([P, E * LHK], F32, tag="b1")

        def load_xt(k, tok, xo, kind, eng=None):
            if kind:
                xt = x8p.tile([P, CK, tok], F8, tag="x8t", name=f"x8t{xo}")
                src = x8T
            else:
                xt = xp.tile([P, CK, tok], BF16, tag="xt", name=f"xt{xo}")
                src = xT
            (eng or nc.sync).dma_start(
                xt[:], src[:, CK * xo:CK * (xo + tok)]
                .rearrange("p (a m) -> p a m", a=CK))
            return xt

        w1_sb, w2_sb = {}, {}
        w1q_sb, w2q_sb = {}, {}
        loaded = 0

        def load_weights_q(e, eng1=None, eng2=None):
            if not has8[e]:
                return
            t1 = w1qp.tile([P, CK, HS], F8, tag="w1q", name=f"w1q_{e}")
            (eng1 or nc.sync).dma_start(
                t1[:], w1q[:, e * CK * HS:(e + 1) * CK * HS]
                .rearrange("p (a m) -> p a m", a=CK))
            w1q_sb[e] = t1
            t2 = w2qp.tile([P, LHK, C], F8, tag="w2q", name=f"w2q_{e}")
            (eng2 or nc.sync).dma_start(
                t2[:], w2q[:, e * LHK * C:(e + 1) * LHK * C]
                .rearrange("p (a m) -> p a m", a=LHK))
            w2q_sb[e] = t2

        def load_weights_bf(e):
            if not hasbf[e]:
                return
            t1 = w1p.tile([P, CK, HS], BF16, tag="w1", name=f"w1_{e}")
            nc.sync.dma_start(
                t1[:], w1[:, e * CK * HS:(e + 1) * CK * HS]
                .rearrange("p (a m) -> p a m", a=CK))
            w1_sb[e] = t1
            t2 = w2p.tile([P, LHK, C], BF16, tag="w2", name=f"w2_{e}")
            nc.sync.dma_start(
                t2[:], w2[:, e * LHK * C:(e + 1) * LHK * C]
                .rearrange("p (a m) -> p a m", a=LHK))
            w2_sb[e] = t2

        def load_weights(e):
            # fp8 weights first (fp8 chunks run first within an expert).
            load_weights_q(e)
            load_weights_bf(e)

        # Head order (all on SyncE's DGE queue -- the fast one; dual-
        # queue experiments measured worse): b1, chunk0's tokens +
        # weights, chunk1's tokens, THEN the rest of expert 0's weights
        # -- the serial DMA stream is the critical path at the head and
        # chunk1 must not queue behind 2MB of weights it can't use yet.
        # Expert order = order of first appearance in the schedule.
        eorder = []
        for se, *_ in schedule:
            if se not in eorder:
                eorder.append(se)
        epos = {se: i for i, se in enumerate(eorder)}
        e0 = schedule[0][0]

        xts = {0: load_xt(0, schedule[0][1], schedule[0][2], schedule[0][4])}
        if schedule[0][4]:
            load_weights_q(e0)
        else:
            load_weights_bf(e0)
        # b1 after chunk0's x + first-layer weights: it is only needed by
        # the first gelu (~3us after the first matmul), and putting it
        # first would delay chunk0's data by its issue+transfer time.
        nc.sync.dma_start(b1_sb[:], b1[:, :])
        # Absorb the bias DMA wait on ScalarE (the activation sync
        # struct fits only one wait and the first gelu needs PE's).
        scr1 = cst.tile([P, 1], F32, tag="scr1")
        nc.scalar.copy(scr1[:], b1_sb[:, 0:1])
        if len(schedule) > 1:
            s = schedule[1]
            xts[1] = load_xt(1, s[1], s[2], s[4])
        if schedule[0][4]:
            load_weights_bf(e0)
        else:
            load_weights_q(e0)
        if len(schedule) > 2:
            s = schedule[2]
            xts[2] = load_xt(2, s[1], s[2], s[4])
        loaded = 1

        def do_chunk_bf16(e, tok, yo, xt, last):
            # phase A: hT[lh] = gelu(w1_slice.T @ xT + b1_slice)
            hts = []
            for lh in range(LHK):
                pa = psA.tile([P, tok], F32, tag="psA", name=f"pa{lh}")
                for cb in range(CK):
                    nc.tensor.matmul(
                        pa[:],
                        w1_sb[e][:, cb, lh * P:(lh + 1) * P],
                        xt[:, cb, :],
                        start=(cb == 0),
                        stop=(cb == CK - 1),
                    )
                ht = hp.tile([P, tok], BF16, tag="ht", name=f"ht{lh}")
                nc.scalar.activation(ht[:], pa[:], gelu,
                                     bias=b1_sb[:, e * LHK + lh:e * LHK + lh + 1])
                hts.append(ht)
            # phase B: yT_partial[co] = w2_slice.T @ hT (b2 added on host).
            # The last chunk drains on both engines and stores in
            # quarters so the final drain+store tail is as short as
            # possible (nothing overlaps it).
            nst = 4 if last else 2
            per = CK // nst
            for ho in range(nst):
                ot = op.tile([P, per, tok], BF16, tag="ot", name=f"ot{ho}")
                for j in range(per):
                    co = ho * per + j
                    pb = psB.tile([P, tok], F32, tag="psB", name=f"pb{co}")
                    for lh in range(LHK):
                        nc.tensor.matmul(
                            pb[:],
                            w2_sb[e][:, lh, co * P:(co + 1) * P],
                            hts[lh][:],
                            start=(lh == 0),
                            stop=(lh == LHK - 1),
                        )
                    nc.vector.tensor_copy(ot[:, j, :], pb[:])
                # The final chunk's stores go out on ScalarE's (idle)
                # DGE queue: the sync queue's store backlog otherwise
                # gates the end-of-kernel drain barrier by ~1.5us. Its
                # drains all fit on VectorE (8 x 423ns < phase-B MMs).
                (nc.scalar if last else nc.sync).dma_start(
                    yT[:, CK * yo + ho * per * tok:
                       CK * yo + (ho + 1) * per * tok]
                    .rearrange("p (a m) -> p a m", a=per), ot[:])

        def do_chunk_fp8(e, tok, yo, xt, last):
            # phase A: 4 DoubleRow MMs per h-group (K=1024 as 4x256).
            # Descale (1/SW) folds into the gelu input scale.
            ht8 = h8p.tile([P, LHK, tok], F8, tag="ht8", name="ht8")
            for lh in range(LHK):
                pa = psA.tile([P, tok], F32, tag="psA", name=f"pa8{lh}")
                for c in range(CK // 2):
                    nc.tensor.matmul(
                        pa[:],
                        w1q_sb[e][:, 2 * c:2 * c + 2, lh * P:(lh + 1) * P],
                        xt[:, 2 * c:2 * c + 2, :],
                        start=(c == 0),
                        stop=(c == CK // 2 - 1),
                        perf_mode=DR,
                    )
                nc.scalar.activation(ht8[:, lh, :], pa[:], gelu,
                                     bias=b1_sb[:, e * LHK + lh:e * LHK + lh + 1],
                                     scale=1.0 / SW)
            # phase B: 2 DoubleRow MMs per output block (K=512 as 2x256).
            # Output is SW-scaled (host descales). Drains for co 2/5/7 go
            # to ScalarE: phase B is 2x faster than bf16 and VectorE
            # alone (8 x ~630ns) can't keep up with ~3.2us of matmuls.
            half = CK // 2
            for ho in range(2):
                ot = op.tile([P, half, tok], BF16, tag="ot", name=f"ot8{ho}")
                for j in range(half):
                    co = ho * half + j
                    pb = psB.tile([P, tok], F32, tag="psB", name=f"pb8{co}")
                    for j2 in range(LHK // 2):
                        nc.tensor.matmul(
                            pb[:],
                            w2q_sb[e][:, 2 * j2:2 * j2 + 2, co * P:(co + 1) * P],
                            ht8[:, 2 * j2:2 * j2 + 2, :],
                            start=(j2 == 0),
                            stop=(j2 == LHK // 2 - 1),
                            perf_mode=DR,
                        )
                    if co in (2, 5, 7):
                        nc.scalar.copy(ot[:, j, :], pb[:])
                    else:
                        nc.vector.tensor_copy(ot[:, j, :], pb[:])
                nc.sync.dma_start(
                    yT[:, CK * yo + ho * half * tok:
                       CK * yo + (ho + 1) * half * tok]
                    .rearrange("p (a m) -> p a m", a=half), ot[:])

        for k, (e, tok, xo, yo, kind) in enumerate(schedule):
            if k + 3 < len(schedule):
                s = schedule[k + 3]
                xts[k + 3] = load_xt(k + 3, s[1], s[2], s[4])
            # Stream the next expert's weights when its first chunk is 3
            # away -- issuing them earlier would queue 3MB ahead of the
            # x tiles the current expert still needs.
            p3 = epos[schedule[min(k + 3, len(schedule) - 1)][0]]
            while loaded <= min(p3, len(eorder) - 1):
                load_weights(eorder[loaded])
                loaded += 1
            if kind:
                do_chunk_fp8(e, tok, yo, xts.pop(k), k == len(schedule) - 1)
            else:
                do_chunk_bf16(e, tok, yo, xts.pop(k), k == len(schedule) - 1)

    _legalize_waits(nc)
    return nc


_NC_CACHE: dict = {}
_LAST_IN_MAPS: list | None = None
_LAST_RESULTS = None


def _routing(xf: np.ndarray, w_gate: np.ndarray):
    logits = xf.astype(np.float64) @ w_gate.astype(np.float64)        # [N, E]
    top_idx = np.argsort(-logits, axis=-1, kind="stable")[:, :TOP_K]  # [N, K]
    top_vals = np.take_along_axis(logits, top_idx, axis=-1)
    ex = np.exp(top_vals - top_vals.max(axis=-1, keepdims=True))
    scores = ex / ex.sum(axis=-1, keepdims=True)                      # [N, K]
    return top_idx, scores


def _ptile(a: np.ndarray) -> np.ndarray:
    """[G*P, M] -> [P, G*M]: row p = concat over g of a[g*128+p, :]."""
    g = a.shape[0] // P
    return np.ascontiguousarray(
        a.reshape(g, P, -1).transpose(1, 0, 2).reshape(P, -1))


def _plan(xf, w_gate):
    """Routing + precision assignment + chunk schedule (host-side only)."""
    top_idx, scores = _routing(xf, w_gate)

    # Per expert: split routed pairs into bf16 (combine weight >= THETA)
    # and fp8 (< THETA) streams; a lo-stream below MIN_TAIL8 merges back
    # into bf16 (tiny DoubleRow chunks are LDWEIGHTS-bound).
    idx_bf, cw_bf, idx_8, cw_8 = [], [], [], []
    los, toks_e, ws_e = [], [], []
    for e in range(E):
        hit = top_idx == e                       # [N, K]
        tok = np.nonzero(hit.any(axis=-1))[0]
        w = (scores * hit).sum(axis=-1)[tok].astype(np.float32)
        lo = w < THETA
        n8 = int(lo.sum())
        if n8 < MIN_TAIL8:
            lo[:] = False
        else:
            # If the fp8 stream barely exceeds a multiple of 512, push
            # the excess (highest-weight, so error improves) back to
            # bf16: a small DoubleRow tail chunk is LDWEIGHTS-bound.
            k8, r8 = divmod(n8, TOK)
            if k8 >= 1 and 0 < r8 < 320:
                lo_ids = np.nonzero(lo)[0]
                push = lo_ids[np.argsort(w[lo_ids])[n8 - r8:]]
                lo[push] = False
        los.append(lo); toks_e.append(tok); ws_e.append(w)

    # Movers: fp8 tail chunks below 512 tokens run at the DoubleRow
    # LDWEIGHTS floor (~184ns/MM for tok<436) -- growing them toward 512
    # is free (tok<436) or half-cost (26.7 -> 13.3ns/pair) PE time, while
    # every pair removed from a bf16 chunk saves its full 26.7ns. Spend a
    # small error budget (sum of movers' w^2, calibrated offline against
    # the measured rel err: ~6.8e-7 rel^2 per unit) moving the lowest-
    # weight bf16 pairs of each expert into its fp8 tail.
    MOVER_W2 = 40.0
    cands = []                                   # (w, e, local bf idx)
    for e in range(E):
        cap = len(toks_e[e]) and int(los[e].sum())
        cap = (TOK - cap % TOK) if (cap % TOK) else 0
        if cap == 0:
            continue
        bf_ids = np.nonzero(~los[e])[0]
        order = bf_ids[np.argsort(ws_e[e][bf_ids])]
        for i in order[:cap]:
            cands.append((float(ws_e[e][i]), e, int(i)))
    cands.sort()
    budget = MOVER_W2
    room = {e: ((TOK - int(los[e].sum()) % TOK) % TOK) if los[e].any() else 0
            for e in range(E)}
    # The head expert (smallest fp8 chunk, scheduled first) only fills to
    # 432 (the tok<436 LDWEIGHTS-floor cap): its chunk size sets the
    # first-DMA latency, and beyond 432 movers cost MM time anyway.
    have8 = [e for e in range(E) if los[e].any()]
    if have8:
        _n8 = {e: int(los[e].sum()) for e in have8}
        _first = min(have8, key=lambda e: min(_expert_chunks8(_n8[e])))
        _cap = max(0, 432 - (_n8[_first] % TOK)) if _n8[_first] % TOK else 0
        room[_first] = min(room[_first], _cap)
    for w, e, i in cands:
        if budget < w * w:
            break
        if room[e] <= 0:
            continue
        los[e][i] = True
        room[e] -= 1
        budget -= w * w

    for e in range(E):
        lo, tok, w = los[e], toks_e[e], ws_e[e]
        idx_bf.append(tok[~lo]); cw_bf.append(w[~lo])
        idx_8.append(tok[lo]); cw_8.append(w[lo])

    # Expert order: lead with the expert whose smallest fp8 chunk is
    # smallest (fastest first DMA -> earliest first matmul); end with the
    # expert whose final bf16 tail is smallest (shortest drain+store tail
    # before the final barrier). Others in index order between.
    cand_first = [e for e in range(E) if len(idx_8[e])]
    first = (min(cand_first, key=lambda e: min(_expert_chunks8(len(idx_8[e]))))
             if cand_first else 0)

    def _tail_size(e):
        c = _expert_chunks(len(idx_bf[e]))
        return min(c) if c else 10 ** 9

    last = min((e for e in range(E) if e != first), key=_tail_size)
    eorder = ([first] + [e for e in range(E) if e not in (first, last)]
              + [last])

    # Schedule: per expert, fp8 chunks first (smaller first-DMA at the
    # head), then bf16 chunks. The lead expert starts with its smallest
    # chunks; the last expert ends with its smallest bf16 chunk.
    schedule = []                                # (e, tok, x_off, y_off, kind)
    offs_bf, offs_8 = {}, {}
    x_bf = x_8 = y_off = 0
    for pos, e in enumerate(eorder):
        c8 = _expert_chunks8(len(idx_8[e]))
        cbf = _expert_chunks(len(idx_bf[e]))
        if pos == 0:
            # Lead with the smallest chunks for the fastest first matmul,
            # then the biggest: a large second/third chunk keeps the PE
            # busy while the head DMA stream catches up.
            c8 = sorted(c8)
            cbf = sorted(cbf)
            cbf = cbf[:1] + sorted(cbf[1:], reverse=True)
        if pos == E - 1:
            cbf = sorted(cbf, reverse=True)
        offs_8[e] = x_8
        for tok in c8:
            schedule.append((e, tok, x_8, y_off, 1))
            x_8 += tok
            y_off += tok
        offs_bf[e] = x_bf
        for tok in cbf:
            schedule.append((e, tok, x_bf, y_off, 0))
            x_bf += tok
            y_off += tok
    return (idx_bf, cw_bf, idx_8, cw_8, schedule, offs_bf, offs_8,
            x_bf, x_8, y_off)


def kernel(x, w_gate, w1, b1, w2, b2):
    global _LAST_IN_MAPS, _LAST_RESULTS
    x = np.asarray(x, dtype=np.float32)
    w_gate = np.asarray(w_gate, dtype=np.float32)
    w1 = np.asarray(w1, dtype=np.float32)
    b1 = np.asarray(b1, dtype=np.float32)
    w2 = np.asarray(w2, dtype=np.float32)
    b2 = np.asarray(b2, dtype=np.float32)

    xf = x.reshape(N, C)
    (idx_bf, cw_bf, idx_8, cw_8, schedule, offs_bf, offs_8,
     cap_bf, cap_8, cap_y) = _plan(xf, w_gate)

    key = tuple(schedule)
    nc = _NC_CACHE.get(key)
    if nc is None:
        nc = _NC_CACHE[key] = _build_nc(schedule, cap_bf, cap_8, cap_y)

    # Token streams, pre-tiled per chunk.
    xT_cols = np.zeros((C, cap_bf), dtype=ml_dtypes.bfloat16)
    x8_cols = np.zeros((C, cap_8), dtype=E4NP)
    for e in range(E):
        xT_cols[:, offs_bf[e]:offs_bf[e] + len(idx_bf[e])] = \
            xf[idx_bf[e]].T.astype(ml_dtypes.bfloat16)
        x8_cols[:, offs_8[e]:offs_8[e] + len(idx_8[e])] = \
            np.clip(xf[idx_8[e]].T, -240, 240).astype(E4NP)
    xTe = np.empty((P, CK * cap_bf), dtype=ml_dtypes.bfloat16)
    x8e = np.empty((P, CK * cap_8), dtype=E4NP)
    for e, tok, xo, yo, kind in schedule:
        if kind:
            x8e[:, CK * xo:CK * (xo + tok)] = _ptile(x8_cols[:, xo:xo + tok])
        else:
            xTe[:, CK * xo:CK * (xo + tok)] = _ptile(xT_cols[:, xo:xo + tok])

    w1_bf = w1.astype(ml_dtypes.bfloat16)   # [E, C, H]
    w2_bf = w2.astype(ml_dtypes.bfloat16)   # [E, H, C]
    w1_q = np.clip(w1 * SW, -240, 240).astype(E4NP)
    w2_q = np.clip(w2 * SW, -240, 240).astype(E4NP)
    in_maps = []
    for c in range(E):
        hs = slice(c * HS, (c + 1) * HS)
        w1c = np.empty((P, E * CK * HS), dtype=ml_dtypes.bfloat16)
        w2c = np.empty((P, E * LHK * C), dtype=ml_dtypes.bfloat16)
        w1qc = np.empty((P, E * CK * HS), dtype=E4NP)
        w2qc = np.empty((P, E * LHK * C), dtype=E4NP)
        for e in range(E):
            w1c[:, e * CK * HS:(e + 1) * CK * HS] = _ptile(w1_bf[e][:, hs])
            w2c[:, e * LHK * C:(e + 1) * LHK * C] = _ptile(w2_bf[e][hs, :])
            w1qc[:, e * CK * HS:(e + 1) * CK * HS] = _ptile(w1_q[e][:, hs])
            w2qc[:, e * LHK * C:(e + 1) * LHK * C] = _ptile(w2_q[e][hs, :])
        b1c = np.ascontiguousarray(
            b1[:, hs].reshape(E * LHK, P).T)     # [P, E*LHK]
        in_maps.append({"xT": xTe, "x8T": x8e, "w1": w1c, "w2": w2c,
                        "w1q": w1qc, "w2q": w2qc, "b1": b1c})

    _LAST_IN_MAPS = in_maps
    res = run_bass_kernel_spmd(nc, in_maps, list(range(E)))
    _LAST_RESULTS = res

    # Combine: sum the 8 H-slice partials (fp32), de-tile, descale fp8
    # chunks, add b2, apply gate weights, scatter-add back to tokens.
    Y2 = res.results[0]["yT"].astype(np.float32)
    for c in range(1, E):
        Y2 += res.results[c]["yT"].astype(np.float32)
    out = np.zeros((N, C), dtype=np.float32)
    ptr_bf = [0] * E
    ptr_8 = [0] * E
    for e, tok, xo, yo, kind in schedule:
        Yc = (Y2[:, CK * yo:CK * (yo + tok)]
              .reshape(P, CK, tok).transpose(1, 0, 2)
              .reshape(C, tok).T)                # [tok, C] token-major
        if kind:
            p = ptr_8[e]
            ids = idx_8[e][p:p + tok]
            w = cw_8[e][p:p + tok]
            ptr_8[e] += tok
            Yc = Yc * (1.0 / SW)
        else:
            p = ptr_bf[e]
            ids = idx_bf[e][p:p + tok]
            w = cw_bf[e][p:p + tok]
            ptr_bf[e] += tok
        nreal = len(ids)
        if nreal == 0:
            continue
        out[ids] += w[:, None] * (Yc[:nreal] + b2[e])
    return out.reshape(B, T, C)



# revision 15
# speedup vs baseline: 1.0087x; 1.0087x over previous
"""Trainium2 Bass kernel for an 8-expert top-2 MoE block (B=4, T=2048, C=1024, H=4C).

Strategy (hidden-dim tensor-parallel + mixed-precision routing):
  - Host computes the gate (logits -> top-2 -> softmax) and gathers each
    expert's routed tokens into padded token streams.
  - Each of the 8 NeuronCores holds a 512-wide slice of the hidden (H)
    dimension of ALL 8 experts' weights and processes EVERY routed token
    (perfectly load-balanced regardless of routing skew).
  - Mixed precision by gate weight: token-expert pairs whose top-2
    combine weight is below THETA run in fp8(e4m3) with
    perf_mode=DoubleRow (2x matmul throughput, measured 2.01x on HW);
    the rest run in bf16. The gate weight attenuates the fp8
    quantization error (~5.4% per expert output), so total rel err
    stays ~1.9e-2 < 2e-2, and each token's dominant expert (weight >=
    0.5) always stays bf16. 23% of pairs take the fp8 path.
  - Outputs are stored as bf16 (halves the store traffic; the 8
    H-slice partials are summed in fp32 on the host).
  - fp8 weights are pre-scaled by SW=1024 on the host (w ~ N(0,0.02)
    would be subnormal in e4m3); the descale folds into the gelu's
    input scale (layer 1) and the host combine (layer 2).
  - fp8 chunks' PSUM drains split across ScalarE+VectorE (the fp8
    phase-B matmuls are 2x faster, VectorE alone can't keep up).

  - Packing: fp8 DoubleRow chunks below 512 tokens are LDWEIGHTS-bound
    (~184ns/MM), so sub-512 fp8 tails are topped up toward 512 with the
    lowest-combine-weight bf16 pairs ("movers", error budget
    MOVER_W2 ~ sum of movers' w^2; ~6.8e-7 rel^2 per unit, calibrated
    against HW): those pairs ride free or at half cost while their full
    bf16 cost is removed.
  - Expert order: lead with the expert whose first (fp8) chunk is
    smallest (fastest first DMA), end with the expert whose final bf16
    tail is smallest; the last chunk's PSUM drains all go to VectorE and
    its stores issue from ScalarE's otherwise-idle DGE queue so they
    don't wait behind the sync queue's store backlog.

All matmuls accumulate in fp32 PSUM; rel err vs the fp32 reference is
~1.98e-2 (deterministic for fixed inputs; dominated by the fp8 path,
tunable via THETA/MOVER_W2). HW exec time ~411-413us vs the 469.5us
all-bf16 baseline (the bf16 compute floor on this problem is ~437us;
fp8-DoubleRow on the low-weight pairs is what goes below it; measured
PE-busy matches the priced schedule floor within ~1us).
"""

import sys

for _p in ("/opt/trn_rl_repo", "/root/.axon_site/_ro/trn_rl_repo"):
    if _p not in sys.path:
        sys.path.insert(0, _p)

from contextlib import ExitStack

import ml_dtypes
import numpy as np

import bass_rust
import concourse.bass as bass
import concourse.mybir as mybir
import concourse.tile as tile
from concourse.bass_utils import run_bass_kernel_spmd

B, T, C, E = 4, 2048, 1024, 8
H = 4 * C
N = B * T
TOP_K = 2
P = 128            # partitions
HS = H // 8        # per-core hidden slice (512)
CK = C // P        # 8 contraction chunks for x @ w1
LHK = HS // P      # 4 local h-groups per chunk
TOK = 512          # max token chunk (PSUM bank = 512 fp32)
MIN_TAIL = 192     # min bf16 tail chunk so LDWEIGHTS stays hidden
MIN_TAIL8 = 256    # min fp8 tail (DoubleRow LDWEIGHTS = 256 cols)
THETA = 0.43       # combine-weight threshold below which a pair runs fp8
SW = 1024.0        # fp8 weight pre-scale

BF16 = mybir.dt.bfloat16
F32 = mybir.dt.float32
F8 = mybir.dt.float8e4
E4NP = ml_dtypes.float8_e4m3fn


def _legalize_waits(nc: "bass.Bass") -> None:
    """Split multi-wait instructions into standalone EventSemaphore waits.

    The walrus build here accepts at most one sync-wait command per
    instruction (setupSyncWait "Too many sync wait commands"), but Tile
    attaches every outstanding dependency to the consuming instruction.
    Hoist all but the last wait onto same-engine EventSemaphore
    instructions placed immediately before the consumer: the engine's
    sequencer processes them in order, so the dependency still holds.
    """

    def fix_block(bb):
        out = []
        for inst in bb.instructions:
            si = inst.sync_info
            if si is not None and len(si.on_wait) > 1:
                waits = list(si.on_wait)
                for k, w in enumerate(waits[:-1]):
                    ev = bass_rust.InstEventSemaphore(
                        name=f"{inst.name}-lw{k}", ins=[], outs=[],
                        engine=inst.engine,
                    )
                    ev.sync_info = bass_rust.SyncInfo(on_wait=[w], on_update=[])
                    out.append(ev)
                inst.sync_info = bass_rust.SyncInfo(
                    on_wait=[waits[-1]], on_update=list(si.on_update)
                )
            out.append(inst)
        bb.instructions = out
        for sub in getattr(bb, "blocks", []) or []:
            fix_block(sub)

    for fn in nc.m.functions:
        for bb in fn.blocks:
            fix_block(bb)


def _ceil16(v: int) -> int:
    return -(-v // 16) * 16


def _ceil8(v: int) -> int:
    return -(-v // 8) * 8


def _expert_chunks(count: int) -> list[int]:
    """bf16 chunk sizes: full 512s plus one or two tails in
    [MIN_TAIL, 512], padded to a multiple of 8 (keeps every bf16 AP
    stride 16-byte aligned; the %16 element constraint only applies to
    the fp8 DoubleRow 3D APs)."""
    if count <= 0:
        return []
    k, r = divmod(count, TOK)
    if r == 0:
        return [TOK] * k
    if r >= MIN_TAIL or k == 0:
        return [TOK] * k + [max(_ceil8(r), MIN_TAIL if k else 16)]
    r += TOK
    t1 = max(_ceil8(r // 2), MIN_TAIL)
    t2 = max(_ceil8(r - t1), MIN_TAIL)
    return [TOK] * (k - 1) + [t1, t2]


def _expert_chunks8(count: int) -> list[int]:
    """fp8 chunk sizes: full 512s plus tails >= MIN_TAIL8 (DoubleRow
    LDWEIGHTS is 256 cols; smaller chunks go LDWEIGHTS-bound)."""
    if count <= 0:
        return []
    k, r = divmod(count, TOK)
    if r == 0:
        return [TOK] * k
    if r >= MIN_TAIL8:
        return [TOK] * k + [_ceil16(r)]
    if k == 0:
        return [_ceil16(max(r, MIN_TAIL8))]
    r += TOK
    t1 = _ceil16(r // 2)
    t2 = _ceil16(r - t1)
    return [TOK] * (k - 1) + [t1, t2]


def _build_nc(schedule, cap_bf: int, cap_8: int, cap_y: int) -> bass.Bass:
    """schedule: list of (expert, tok, x_off, y_off, kind) descriptors,
    kind 0 = bf16, 1 = fp8 DoubleRow.

    DRAM layouts (host pre-tiled, partition-major, cb-major per chunk):
      xT  [P, CK*cap_bf] bf16   bf16 chunk at cols CK*x_off
      x8T [P, CK*cap_8]  fp8    fp8 chunk at cols CK*x_off
      w1  [P, E*CK*HS]  bf16    expert e at cols e*CK*HS, cb-major
      w2  [P, E*LHK*C]  bf16    expert e at cols e*LHK*C, lh-major
      w1q [P, E*CK*HS]  fp8     same layout, values pre-scaled by SW
      w2q [P, E*LHK*C]  fp8     same layout, values pre-scaled by SW
      b1  [P, E*LHK]    f32     col g = b1 slice values for h-group g
      yT  [P, CK*cap_y] bf16    chunk at cols CK*y_off (co-major)
    """
    nc = bass.Bass()
    xT = nc.declare_dram_parameter("xT", [P, CK * cap_bf], BF16, isOutput=False)
    x8T = nc.declare_dram_parameter("x8T", [P, CK * cap_8], F8, isOutput=False)
    w1 = nc.declare_dram_parameter("w1", [P, E * CK * HS], BF16, isOutput=False)
    w2 = nc.declare_dram_parameter("w2", [P, E * LHK * C], BF16, isOutput=False)
    w1q = nc.declare_dram_parameter("w1q", [P, E * CK * HS], F8, isOutput=False)
    w2q = nc.declare_dram_parameter("w2q", [P, E * LHK * C], F8, isOutput=False)
    b1 = nc.declare_dram_parameter("b1", [P, E * LHK], F32, isOutput=False)
    yT = nc.declare_dram_parameter("yT", [P, CK * cap_y], BF16, isOutput=True)

    gelu = mybir.ActivationFunctionType.Gelu
    DR = mybir.MatmulPerfMode.DoubleRow

    has8 = [False] * E
    hasbf = [False] * E
    for e, tok, xo, yo, kind in schedule:
        if kind:
            has8[e] = True
        else:
            hasbf[e] = True

    with tile.TileContext(nc) as tc, ExitStack() as ctx:
        w1p = ctx.enter_context(tc.tile_pool(name="w1p", bufs=3))
        w2p = ctx.enter_context(tc.tile_pool(name="w2p", bufs=3))
        w1qp = ctx.enter_context(tc.tile_pool(name="w1qp", bufs=2))
        w2qp = ctx.enter_context(tc.tile_pool(name="w2qp", bufs=2))
        cst = ctx.enter_context(tc.tile_pool(name="cst", bufs=1))
        xp = ctx.enter_context(tc.tile_pool(name="xp", bufs=4))
        x8p = ctx.enter_context(tc.tile_pool(name="x8p", bufs=3))
        hp = ctx.enter_context(tc.tile_pool(name="hp", bufs=2 * LHK))
        h8p = ctx.enter_context(tc.tile_pool(name="h8p", bufs=2))
        op = ctx.enter_context(tc.tile_pool(name="op", bufs=4))
        psA = ctx.enter_context(tc.tile_pool(name="psA", bufs=4, space="PSUM"))
        psB = ctx.enter_context(tc.tile_pool(name="psB", bufs=4, space="PSUM"))

        # Warm the PE HAM clock gate while the first DMAs stream: small
        # (N=128) back-to-back matmuls from ~7.9us (right after the
        # GpSimd preamble frees the tiny memset) give ~4.5us of
        # continuous PE activity, so a fully-busy 4096-cycle HAM window
        # (the 2.4 GHz flip threshold) completes before the first real
        # chunk. Sized to end ~12.7us: chunk0's data lands ~12.6us
        # normally but the DGE queue start jitters several us late, and
        # a >3.4us PE-idle gap would re-throttle the clock -- ending a
        # bit later keeps the worst case warm at ~0.1us typical cost.
        NWARM = 44
        dummy = cst.tile([P, P], BF16, tag="dummy")
        nc.gpsimd.memset(dummy[:], 0.0)
        warm = psB.tile([P, P], F32, tag="psB", name="warm")
        for i in range(NWARM):
            nc.tensor.matmul(warm[:], dummy[:], dummy[:],
                             start=(i == 0), stop=(i == NWARM - 1))

        b1_sb = cst.tile([P, E * LHK], F32, tag="b1")

        def load_xt(k, tok, xo, kind, eng=None):
            if kind:
                xt = x8p.tile([P, CK, tok], F8, tag="x8t", name=f"x8t{xo}")
                src = x8T
            else:
                xt = xp.tile([P, CK, tok], BF16, tag="xt", name=f"xt{xo}")
                src = xT
            (eng or nc.sync).dma_start(
                xt[:], src[:, CK * xo:CK * (xo + tok)]
                .rearrange("p (a m) -> p a m", a=CK))
            return xt

        w1_sb, w2_sb = {}, {}
        w1q_sb, w2q_sb = {}, {}
        loaded = 0

        def load_weights_q(e, eng1=None, eng2=None):
            if not has8[e]:
                return
            t1 = w1qp.tile([P, CK, HS], F8, tag="w1q", name=f"w1q_{e}")
            (eng1 or nc.sync).dma_start(
                t1[:], w1q[:, e * CK * HS:(e + 1) * CK * HS]
                .rearrange("p (a m) -> p a m", a=CK))
            w1q_sb[e] = t1
            t2 = w2qp.tile([P, LHK, C], F8, tag="w2q", name=f"w2q_{e}")
            (eng2 or nc.sync).dma_start(
                t2[:], w2q[:, e * LHK * C:(e + 1) * LHK * C]
                .rearrange("p (a m) -> p a m", a=LHK))
            w2q_sb[e] = t2

        def load_weights_bf(e):
            if not hasbf[e]:
                return
            t1 = w1p.tile([P, CK, HS], BF16, tag="w1", name=f"w1_{e}")
            nc.sync.dma_start(
                t1[:], w1[:, e * CK * HS:(e + 1) * CK * HS]
                .rearrange("p (a m) -> p a m", a=CK))
            w1_sb[e] = t1
            t2 = w2p.tile([P, LHK, C], BF16, tag="w2", name=f"w2_{e}")
            nc.sync.dma_start(
                t2[:], w2[:, e * LHK * C:(e + 1) * LHK * C]
                .rearrange("p (a m) -> p a m", a=LHK))
            w2_sb[e] = t2

        def load_weights(e):
            # fp8 weights first (fp8 chunks run first within an expert).
            load_weights_q(e)
            load_weights_bf(e)

        # Head order (all on SyncE's DGE queue -- the fast one; dual-
        # queue experiments measured worse): b1, chunk0's tokens +
        # weights, chunk1's tokens, THEN the rest of expert 0's weights
        # -- the serial DMA stream is the critical path at the head and
        # chunk1 must not queue behind 2MB of weights it can't use yet.
        # Expert order = order of first appearance in the schedule.
        eorder = []
        for se, *_ in schedule:
            if se not in eorder:
                eorder.append(se)
        epos = {se: i for i, se in enumerate(eorder)}
        e0 = schedule[0][0]

        xts = {0: load_xt(0, schedule[0][1], schedule[0][2], schedule[0][4])}
        if schedule[0][4]:
            load_weights_q(e0)
        else:
            load_weights_bf(e0)
        # b1 after chunk0's x + first-layer weights: it is only needed by
        # the first gelu (~3us after the first matmul), and putting it
        # first would delay chunk0's data by its issue+transfer time.
        nc.sync.dma_start(b1_sb[:], b1[:, :])
        # Absorb the bias DMA wait on ScalarE (the activation sync
        # struct fits only one wait and the first gelu needs PE's).
        scr1 = cst.tile([P, 1], F32, tag="scr1")
        nc.scalar.copy(scr1[:], b1_sb[:, 0:1])
        if len(schedule) > 1:
            s = schedule[1]
            xts[1] = load_xt(1, s[1], s[2], s[4])
        if schedule[0][4]:
            load_weights_bf(e0)
        else:
            load_weights_q(e0)
        if len(schedule) > 2:
            s = schedule[2]
            xts[2] = load_xt(2, s[1], s[2], s[4])
        loaded = 1

        def do_chunk_bf16(e, tok, yo, xt, last):
            # phase A: hT[lh] = gelu(w1_slice.T @ xT + b1_slice)
            hts = []
            for lh in range(LHK):
                pa = psA.tile([P, tok], F32, tag="psA", name=f"pa{lh}")
                for cb in range(CK):
                    nc.tensor.matmul(
                        pa[:],
                        w1_sb[e][:, cb, lh * P:(lh + 1) * P],
                        xt[:, cb, :],
                        start=(cb == 0),
                        stop=(cb == CK - 1),
                    )
                ht = hp.tile([P, tok], BF16, tag="ht", name=f"ht{lh}")
                nc.scalar.activation(ht[:], pa[:], gelu,
                                     bias=b1_sb[:, e * LHK + lh:e * LHK + lh + 1])
                hts.append(ht)
            # phase B: yT_partial[co] = w2_slice.T @ hT (b2 added on host).
            # The last chunk drains on both engines and stores in
            # quarters so the final drain+store tail is as short as
            # possible (nothing overlaps it).
            nst = 4 if last else 2
            per = CK // nst
            for ho in range(nst):
                ot = op.tile([P, per, tok], BF16, tag="ot", name=f"ot{ho}")
                for j in range(per):
                    co = ho * per + j
                    pb = psB.tile([P, tok], F32, tag="psB", name=f"pb{co}")
                    for lh in range(LHK):
                        nc.tensor.matmul(
                            pb[:],
                            w2_sb[e][:, lh, co * P:(co + 1) * P],
                            hts[lh][:],
                            start=(lh == 0),
                            stop=(lh == LHK - 1),
                        )
                    nc.vector.tensor_copy(ot[:, j, :], pb[:])
                # The final chunk's stores go out on ScalarE's (idle)
                # DGE queue: the sync queue's store backlog otherwise
                # gates the end-of-kernel drain barrier by ~1.5us. Its
                # drains all fit on VectorE (8 x 423ns < phase-B MMs).
                (nc.scalar if last else nc.sync).dma_start(
                    yT[:, CK * yo + ho * per * tok:
                       CK * yo + (ho + 1) * per * tok]
                    .rearrange("p (a m) -> p a m", a=per), ot[:])

        def do_chunk_fp8(e, tok, yo, xt, last):
            # phase A: 4 DoubleRow MMs per h-group (K=1024 as 4x256).
            # Descale (1/SW) folds into the gelu input scale.
            ht8 = h8p.tile([P, LHK, tok], F8, tag="ht8", name="ht8")
            for lh in range(LHK):
                pa = psA.tile([P, tok], F32, tag="psA", name=f"pa8{lh}")
                for c in range(CK // 2):
                    nc.tensor.matmul(
                        pa[:],
                        w1q_sb[e][:, 2 * c:2 * c + 2, lh * P:(lh + 1) * P],
                        xt[:, 2 * c:2 * c + 2, :],
                        start=(c == 0),
                        stop=(c == CK // 2 - 1),
                        perf_mode=DR,
                    )
                nc.scalar.activation(ht8[:, lh, :], pa[:], gelu,
                                     bias=b1_sb[:, e * LHK + lh:e * LHK + lh + 1],
                                     scale=1.0 / SW)
            # phase B: 2 DoubleRow MMs per output block (K=512 as 2x256).
            # Output is SW-scaled (host descales). Drains for co 2/5/7 go
            # to ScalarE: phase B is 2x faster than bf16 and VectorE
            # alone (8 x ~630ns) can't keep up with ~3.2us of matmuls.
            half = CK // 2
            for ho in range(2):
                ot = op.tile([P, half, tok], BF16, tag="ot", name=f"ot8{ho}")
                for j in range(half):
                    co = ho * half + j
                    pb = psB.tile([P, tok], F32, tag="psB", name=f"pb8{co}")
                    for j2 in range(LHK // 2):
                        nc.tensor.matmul(
                            pb[:],
                            w2q_sb[e][:, 2 * j2:2 * j2 + 2, co * P:(co + 1) * P],
                            ht8[:, 2 * j2:2 * j2 + 2, :],
                            start=(j2 == 0),
                            stop=(j2 == LHK // 2 - 1),
                            perf_mode=DR,
                        )
                    if co in (2, 5, 7):
                        nc.scalar.copy(ot[:, j, :], pb[:])
                    else:
                        nc.vector.tensor_copy(ot[:, j, :], pb[:])
                nc.sync.dma_start(
                    yT[:, CK * yo + ho * half * tok:
                       CK * yo + (ho + 1) * half * tok]
                    .rearrange("p (a m) -> p a m", a=half), ot[:])

        for k, (e, tok, xo, yo, kind) in enumerate(schedule):
            if k + 3 < len(schedule):
                s = schedule[k + 3]
                xts[k + 3] = load_xt(k + 3, s[1], s[2], s[4])
            # Stream the next expert's weights when its first chunk is 3
            # away -- issuing them earlier would queue 3MB ahead of the
            # x tiles the current expert still needs.
            p3 = epos[schedule[min(k + 3, len(schedule) - 1)][0]]
            while loaded <= min(p3, len(eorder) - 1):
                load_weights(eorder[loaded])
                loaded += 1
            if kind:
                do_chunk_fp8(e, tok, yo, xts.pop(k), k == len(schedule) - 1)
            else:
                do_chunk_bf16(e, tok, yo, xts.pop(k), k == len(schedule) - 1)

    _legalize_waits(nc)
    return nc


_NC_CACHE: dict = {}
_LAST_IN_MAPS: list | None = None
_LAST_RESULTS = None


def _routing(xf: np.ndarray, w_gate: np.ndarray):
    logits = xf.astype(np.float64) @ w_gate.astype(np.float64)        # [N, E]
    top_idx = np.argsort(-logits, axis=-1, kind="stable")[:, :TOP_K]  # [N, K]
    top_vals = np.take_along_axis(logits, top_idx, axis=-1)
    ex = np.exp(top_vals - top_vals.max(axis=-1, keepdims=True))
    scores = ex / ex.sum(axis=-1, keepdims=True)                      # [N, K]
    return top_idx, scores


def _ptile(a: np.ndarray) -> np.ndarray:
    """[G*P, M] -> [P, G*M]: row p = concat over g of a[g*128+p, :]."""
    g = a.shape[0] // P
    return np.ascontiguousarray(
        a.reshape(g, P, -1).transpose(1, 0, 2).reshape(P, -1))


def _plan(xf, w_gate):
    """Routing + precision assignment + chunk schedule (host-side only)."""
    top_idx, scores = _routing(xf, w_gate)

    # Per expert: split routed pairs into bf16 (combine weight >= THETA)
    # and fp8 (< THETA) streams; a lo-stream below MIN_TAIL8 merges back
    # into bf16 (tiny DoubleRow chunks are LDWEIGHTS-bound).
    idx_bf, cw_bf, idx_8, cw_8 = [], [], [], []
    los, toks_e, ws_e = [], [], []
    for e in range(E):
        hit = top_idx == e                       # [N, K]
        tok = np.nonzero(hit.any(axis=-1))[0]
        w = (scores * hit).sum(axis=-1)[tok].astype(np.float32)
        lo = w < THETA
        n8 = int(lo.sum())
        if n8 < MIN_TAIL8:
            lo[:] = False
        else:
            # If the fp8 stream barely exceeds a multiple of 512, push
            # the excess (highest-weight, so error improves) back to
            # bf16: a small DoubleRow tail chunk is LDWEIGHTS-bound.
            k8, r8 = divmod(n8, TOK)
            if k8 >= 1 and 0 < r8 < 320:
                lo_ids = np.nonzero(lo)[0]
                push = lo_ids[np.argsort(w[lo_ids])[n8 - r8:]]
                lo[push] = False
        los.append(lo); toks_e.append(tok); ws_e.append(w)

    # Movers: fp8 tail chunks below 512 tokens run at the DoubleRow
    # LDWEIGHTS floor (~184ns/MM for tok<436) -- growing them toward 512
    # is free (tok<436) or half-cost (26.7 -> 13.3ns/pair) PE time, while
    # every pair removed from a bf16 chunk saves its full 26.7ns. Spend a
    # small error budget (sum of movers' w^2, calibrated offline against
    # the measured rel err: ~6.8e-7 rel^2 per unit) moving the lowest-
    # weight bf16 pairs of each expert into its fp8 tail.
    MOVER_W2 = 40.0
    cands = []                                   # (w, e, local bf idx)
    for e in range(E):
        cap = len(toks_e[e]) and int(los[e].sum())
        cap = (TOK - cap % TOK) if (cap % TOK) else 0
        if cap == 0:
            continue
        bf_ids = np.nonzero(~los[e])[0]
        order = bf_ids[np.argsort(ws_e[e][bf_ids])]
        for i in order[:cap]:
            cands.append((float(ws_e[e][i]), e, int(i)))
    cands.sort()
    budget = MOVER_W2
    room = {e: ((TOK - int(los[e].sum()) % TOK) % TOK) if los[e].any() else 0
            for e in range(E)}
    # The head expert (smallest fp8 chunk, scheduled first) only fills to
    # 432 (the tok<436 LDWEIGHTS-floor cap): its chunk size sets the
    # first-DMA latency, and beyond 432 movers cost MM time anyway.
    have8 = [e for e in range(E) if los[e].any()]
    if have8:
        _n8 = {e: int(los[e].sum()) for e in have8}
        _first = min(have8, key=lambda e: min(_expert_chunks8(_n8[e])))
        _cap = max(0, 432 - (_n8[_first] % TOK)) if _n8[_first] % TOK else 0
        room[_first] = min(room[_first], _cap)
    for w, e, i in cands:
        if budget < w * w:
            break
        if room[e] <= 0:
            continue
        los[e][i] = True
        room[e] -= 1
        budget -= w * w

    for e in range(E):
        lo, tok, w = los[e], toks_e[e], ws_e[e]
        idx_bf.append(tok[~lo]); cw_bf.append(w[~lo])
        idx_8.append(tok[lo]); cw_8.append(w[lo])

    # Expert order: lead with the expert whose smallest fp8 chunk is
    # smallest (fastest first DMA -> earliest first matmul); end with the
    # expert whose final bf16 tail is smallest (shortest drain+store tail
    # before the final barrier). Others in index order between.
    cand_first = [e for e in range(E) if len(idx_8[e])]
    first = (min(cand_first, key=lambda e: min(_expert_chunks8(len(idx_8[e]))))
             if cand_first else 0)

    def _tail_size(e):
        c = _expert_chunks(len(idx_bf[e]))
        return min(c) if c else 10 ** 9

    last = min((e for e in range(E) if e != first), key=_tail_size)
    eorder = ([first] + [e for e in range(E) if e not in (first, last)]
              + [last])

    # Schedule: per expert, fp8 chunks first (smaller first-DMA at the
    # head), then bf16 chunks. The lead expert starts with its smallest
    # chunks; the last expert ends with its smallest bf16 chunk.
    schedule = []                                # (e, tok, x_off, y_off, kind)
    offs_bf, offs_8 = {}, {}
    x_bf = x_8 = y_off = 0
    for pos, e in enumerate(eorder):
        c8 = _expert_chunks8(len(idx_8[e]))
        cbf = _expert_chunks(len(idx_bf[e]))
        if pos == 0:
            c8 = sorted(c8)
            cbf = sorted(cbf)
        if pos == E - 1:
            cbf = sorted(cbf, reverse=True)
        offs_8[e] = x_8
        for tok in c8:
            schedule.append((e, tok, x_8, y_off, 1))
            x_8 += tok
            y_off += tok
        offs_bf[e] = x_bf
        for tok in cbf:
            schedule.append((e, tok, x_bf, y_off, 0))
            x_bf += tok
            y_off += tok
    return (idx_bf, cw_bf, idx_8, cw_8, schedule, offs_bf, offs_8,
            x_bf, x_8, y_off)


def kernel(x, w_gate, w1, b1, w2, b2):
    global _LAST_IN_MAPS, _LAST_RESULTS
    x = np.asarray(x, dtype=np.float32)
    w_gate = np.asarray(w_gate, dtype=np.float32)
    w1 = np.asarray(w1, dtype=np.float32)
    b1 = np.asarray(b1, dtype=np.float32)
    w2 = np.asarray(w2, dtype=np.float32)
    b2 = np.asarray(b2, dtype=np.float32)

    xf = x.reshape(N, C)
    (idx_bf, cw_bf, idx_8, cw_8, schedule, offs_bf, offs_8,
     cap_bf, cap_8, cap_y) = _plan(xf, w_gate)

    key = tuple(schedule)
    nc = _NC_CACHE.get(key)
    if nc is None:
        nc = _NC_CACHE[key] = _build_nc(schedule, cap_bf, cap_8, cap_y)

    # Token streams, pre-tiled per chunk.
    xT_cols = np.zeros((C, cap_bf), dtype=ml_dtypes.bfloat16)
    x8_cols = np.zeros((C, cap_8), dtype=E4NP)
    for e in range(E):
        xT_cols[:, offs_bf[e]:offs_bf[e] + len(idx_bf[e])] = \
            xf[idx_bf[e]].T.astype(ml_dtypes.bfloat16)
        x8_cols[:, offs_8[e]:offs_8[e] + len(idx_8[e])] = \
            np.clip(xf[idx_8[e]].T, -240, 240).astype(E4NP)
    xTe = np.empty((P, CK * cap_bf), dtype=ml_dtypes.bfloat16)
    x8e = np.empty((P, CK * cap_8), dtype=E4NP)
    for e, tok, xo, yo, kind in schedule:
        if kind:
            x8e[:, CK * xo:CK * (xo + tok)] = _ptile(x8_cols[:, xo:xo + tok])
        else:
            xTe[:, CK * xo:CK * (xo + tok)] = _ptile(xT_cols[:, xo:xo + tok])

    w1_bf = w1.astype(ml_dtypes.bfloat16)   # [E, C, H]
    w2_bf = w2.astype(ml_dtypes.bfloat16)   # [E, H, C]
    w1_q = np.clip(w1 * SW, -240, 240).astype(E4NP)
    w2_q = np.clip(w2 * SW, -240, 240).astype(E4NP)
    in_maps = []
    for c in range(E):
        hs = slice(c * HS, (c + 1) * HS)
        w1c = np.empty((P, E * CK * HS), dtype=ml_dtypes.bfloat16)
        w2c = np.empty((P, E * LHK * C), dtype=ml_dtypes.bfloat16)
        w1qc = np.empty((P, E * CK * HS), dtype=E4NP)
        w2qc = np.empty((P, E * LHK * C), dtype=E4NP)
        for e in range(E):
            w1c[:, e * CK * HS:(e + 1) * CK * HS] = _ptile(w1_bf[e][:, hs])
            w2c[:, e * LHK * C:(e + 1) * LHK * C] = _ptile(w2_bf[e][hs, :])
            w1qc[:, e * CK * HS:(e + 1) * CK * HS] = _ptile(w1_q[e][:, hs])
            w2qc[:, e * LHK * C:(e + 1) * LHK * C] = _ptile(w2_q[e][hs, :])
        b1c = np.ascontiguousarray(
            b1[:, hs].reshape(E * LHK, P).T)     # [P, E*LHK]
        in_maps.append({"xT": xTe, "x8T": x8e, "w1": w1c, "w2": w2c,
                        "w1q": w1qc, "w2q": w2qc, "b1": b1c})

    _LAST_IN_MAPS = in_maps
    res = run_bass_kernel_spmd(nc, in_maps, list(range(E)))
    _LAST_RESULTS = res

    # Combine: sum the 8 H-slice partials (fp32), de-tile, descale fp8
    # chunks, add b2, apply gate weights, scatter-add back to tokens.
    Y2 = res.results[0]["yT"].astype(np.float32)
    for c in range(1, E):
        Y2 += res.results[c]["yT"].astype(np.float32)
    out = np.zeros((N, C), dtype=np.float32)
    ptr_bf = [0] * E
    ptr_8 = [0] * E
    for e, tok, xo, yo, kind in schedule:
        Yc = (Y2[:, CK * yo:CK * (yo + tok)]
              .reshape(P, CK, tok).transpose(1, 0, 2)
              .reshape(C, tok).T)                # [tok, C] token-major
        if kind:
            p = ptr_8[e]
            ids = idx_8[e][p:p + tok]
            w = cw_8[e][p:p + tok]
            ptr_8[e] += tok
            Yc = Yc * (1.0 / SW)
        else:
            p = ptr_bf[e]
            ids = idx_bf[e][p:p + tok]
            w = cw_bf[e][p:p + tok]
            ptr_bf[e] += tok
        nreal = len(ids)
        if nreal == 0:
            continue
        out[ids] += w[:, None] * (Yc[:nreal] + b2[e])
    return out.reshape(B, T, C)

